# revision 28
# baseline (speedup 1.0000x reference)
"""Distributed embedding-lookup kernel for 8 TRN2 NeuronCores (Bass/Tile).

Computes, for full inputs:
    word_sum = sum(word_matrix[context_ids], axis=1)        # [B, D]
    inputs   = paragraph_matrix[doc_ids] + word_sum         # [B, D]
    out_cols = outputs[:, sample_ids]                       # [D, B, S]
    logits   = einsum("bd,dbs->bs", inputs, out_cols)       # [B, S]

Strategy (SPMD, one NEFF on 8 cores; per-core variation lives in input data):
  Phase A (batch-sharded, 2048 rows/core): all 9 embedding-row fetches per
    batch element (1 doc + 8 ctx) are gathered via windowed dma_gather
    (int16 indices limited to 32767 -> 4 windows of 25000 rows per table),
    written compacted to a DRAM stage buffer, re-gathered in
    (entry-major, batch-minor) slot order (stage row ids < 32767), then
    reduced over the 9 entries with strided DVE adds -> inputs [2048, 128].
  AllGather inputs across cores -> [16384, 128] per core.
  Phase B (vocab-sharded: core k owns outputs[:, 12500k:12500(k+1)]):
    PE-transpose the slice to T [12544, 128] in DRAM; dma_gather T rows by
    local sample column and inputs rows by sample batch id; DVE mul +
    free-dim reduce gives one dot product per sample; host scatters values
    into the [16384, 6] output.
All index lists / stage positions / scatter maps are precomputed on host
(pure index arithmetic; all bulk data movement happens on device).
"""

import sys
import types

import numpy as np

# ---------------------------------------------------------------------------
# problem constants (hardcoded per contract)
B = 16384
D = 128
CTX = 8
S = 6
V = 100000
N_CORES = 8
BL = B // N_CORES              # 2048 batch rows per core
RV = V // N_CORES              # 12500 outputs columns per core
WIN = 25000                    # gather window (int16 indices must be <= 32767)
NWIN = V // WIN                # 4 windows per table
NP_DOC = 384                   # per-(half,window) doc list (avg 256, seed max 299)
NP_CTX = 2304                  # per-(half,window) ctx list (avg 2048, seed max 2081)
NSTAGE = NWIN * (NP_DOC + NP_CTX)   # 10752 stage rows per half
NPB = 12800                    # padded per-core samples (avg 12288; 6400/half, seed max 6261)
TCOLS = 12544                  # outputs cols padded to 98*128 for transpose
IDX_COLS = (2 * NWIN * (NP_DOC // 16) + 2 * NWIN * (NP_CTX // 16)
            + 18 * (BL // 2 // 16) + 8 * (NPB // 4 // 16))  # 4096

_nc_cache = None


def _install_ntff_hook():
    """antenv.axon_hooks is absent from this image; inject it so
    run_bass_kernel_spmd(trace=True) can capture NTFF profiles."""
    if "antenv.axon_hooks" in sys.modules:
        return
    mod = types.ModuleType("antenv.axon_hooks")
    mod._hook = None
    mod.set_axon_ntff_profile_hook = lambda h: setattr(mod, "_hook", h)
    mod.get_axon_ntff_profile_hook = lambda: mod._hook
    sys.modules["antenv.axon_hooks"] = mod
    try:
        import antenv
        antenv.axon_hooks = mod
        from trn_agent_boot.trn_boot import _ntff_profile_via_ctypes
        mod.set_axon_ntff_profile_hook(
            _ntff_profile_via_ctypes("/opt/axon/libaxon_pjrt.so"))
    except Exception:
        pass


def _patch_swdge_lane_assignment():
    """Tile round-robins SWDGE DMA completion sems over all 8 DMASW lanes,
    but the runtime locks each sem lane to the first SWDGE queue that
    increments it — mixed-queue kernels then abort.  Pin queue-tagged SWDGE
    ops (dma_gather et al.) to lane == queue_num, and round-robin untagged
    SWDGE DMAs over lanes 4..7 so the two sets never share a lane."""
    import concourse.tile_sem_assignment as tsa
    import concourse.mybir as mybir
    from concourse import bass_isa

    if getattr(tsa.TileClockTick, "_lane_patch", False):
        return
    orig = tsa.TileClockTick._assign_tick

    def _assign_tick(self, inst):
        if (
            isinstance(inst, tsa.DMAInst)
            and not isinstance(inst, bass_isa.UserSyncedRemoteDMADescs)
            and inst.engine == mybir.EngineType.Pool
        ):
            qn = getattr(inst, "queue_num", None)
            if isinstance(qn, int) and 0 <= qn <= 3:
                lane = qn
            else:
                lane = 4 + self.next_sw_dma_idx % 4
                self.next_sw_dma_idx += 1
            proc = tsa.PROC_NAME_TO_IDX[f"DMASW{lane}"]
            inst.bass_scheduled_tick = self.global_clock.advance(proc)
            inst.bass_scheduled_proc = proc
            inst.bass_scheduled_scope = self.scope_name
            self._proc_insts[self.root_scope_name][proc].append(inst)
            eng_proc = tsa.ENGINE_TO_IDX[inst.engine]
            if getattr(inst, "gen_mode", 0) == 1 and proc != eng_proc:
                eng_tick = self.global_clock.advance(eng_proc)
                self.tc.prep_eng_ticks[inst.name] = (eng_proc, eng_tick)
                self._prep_eng_names[self.root_scope_name].append(inst.name)
            return
        return orig(self, inst)

    tsa.TileClockTick._assign_tick = _assign_tick
    tsa.TileClockTick._lane_patch = True


def _build_nc():
    import concourse.bacc as bacc
    import concourse.mybir as mybir
    import concourse.tile as tile

    _patch_swdge_lane_assignment()

    f32 = mybir.dt.float32
    i16 = mybir.dt.int16

    nc = bacc.Bacc("TRN2", target_bir_lowering=False, debug=False,
                   num_devices=N_CORES, num_swdge_queues=4)

    idx_d = nc.dram_tensor("idx", [128, IDX_COLS], i16, kind="ExternalInput")
    ptab = nc.dram_tensor("ptab", [V, D], f32, kind="ExternalInput")
    wtab = nc.dram_tensor("wtab", [V, D], f32, kind="ExternalInput")
    ocols = nc.dram_tensor("ocols", [128, TCOLS], f32, kind="ExternalInput")
    ident = nc.dram_tensor("ident", [128, 128], f32, kind="ExternalInput")
    vals_d = nc.dram_tensor("vals", [128, NPB // 128], f32,
                            kind="ExternalOutput")

    with tile.TileContext(nc) as tc:
        with (
            tc.tile_pool(name="dram", bufs=1, space="DRAM") as dpool,
            tc.tile_pool(name="const", bufs=1) as cpool,
            tc.tile_pool(name="acc", bufs=1) as apool,
            tc.tile_pool(name="vals", bufs=1) as vpool,
        ):
            stage0 = dpool.tile([NSTAGE, D], f32)
            stage1 = dpool.tile([NSTAGE, D], f32)
            stages = [stage0, stage1]
            tdram = dpool.tile([TCOLS, D], f32)
            inb = dpool.tile([BL, D], f32)
            agh0 = dpool.tile([B // 2, D], f32)
            agh1 = dpool.tile([B // 2, D], f32)
            agh = [agh0, agh1]

            import concourse.mybir as _mb

            idx_sb = cpool.tile([128, IDX_COLS], i16)
            nc.sync.dma_start(idx_sb[:], idx_d[:])
            ident_sb = cpool.tile([128, 128], f32)
            nc.sync.dma_start(ident_sb[:], ident[:])
            ok_sb = cpool.tile([128, TCOLS], f32)
            nc.scalar.dma_start(ok_sb[:], ocols[:])

            # ---- Phase A, by batch-half: windowed gathers -> stage_h,
            # slot-order regather + entry reduction; each half's AllGather
            # fires while the other half is still gathering.
            HB = BL // 2                  # 1024 rows per half
            col = 0
            qn = 0
            acc = apool.tile([128, (BL // 128) * D], f32)
            acc3 = acc[:].rearrange("p (t d) -> p t d", d=D)
            with (
                tc.tile_pool(name="g1doc", bufs=4) as gdoc,
                tc.tile_pool(name="g1ctx", bufs=6) as gctx,
                tc.tile_pool(name="g2", bufs=6) as g2pool,
            ):
                for h in range(2):
                    stage = stages[h]
                    srow = 0
                    qn = 0
                    for w in range(NWIN):
                        gt = gdoc.tile([128, NP_DOC // 128 * D], f32)
                        gt3 = gt[:].rearrange("p (c d) -> p c d", d=D)
                        nc.gpsimd.dma_gather(
                            out_ap=gt3,
                            in_ap=ptab[w * WIN:(w + 1) * WIN, :],
                            idxs_ap=idx_sb[:, col:col + NP_DOC // 16],
                            num_idxs=NP_DOC,
                            num_idxs_reg=NP_DOC,
                            elem_size=D,
                            queue_num=2 * h + qn % 2,
                            single_packet=False,
                        )
                        nc.sync.dma_start(
                            stage[:][srow:srow + NP_DOC, :]
                            .rearrange("(p c) d -> p c d", p=128),
                            gt3)
                        col += NP_DOC // 16
                        srow += NP_DOC
                        qn += 1
                    CQ = NP_CTX // 2      # 1152 per sub-call
                    for w in range(NWIN):
                        for q in range(2):
                            gt = gctx.tile([128, CQ // 128 * D], f32)
                            gt3 = gt[:].rearrange("p (c d) -> p c d", d=D)
                            nc.gpsimd.dma_gather(
                                out_ap=gt3,
                                in_ap=wtab[w * WIN:(w + 1) * WIN, :],
                                idxs_ap=idx_sb[:, col:col + CQ // 16],
                                num_idxs=CQ,
                                num_idxs_reg=CQ,
                                elem_size=D,
                                queue_num=2 * h + qn % 2,
                                single_packet=False,
                            )
                            nc.sync.dma_start(
                                stage[:][srow:srow + CQ, :]
                                .rearrange("(p c) d -> p c d", p=128),
                                gt3)
                            col += CQ // 16
                            srow += CQ
                            qn += 1
                    hv = acc3[:, h * (HB // 128):(h + 1) * (HB // 128)]
                    for e in range(9):
                        g2t = g2pool.tile([128, (HB // 128) * D], f32)
                        g2v = g2t[:].rearrange("p (t d) -> p t d", d=D)
                        nc.gpsimd.dma_gather(
                            out_ap=g2v,
                            in_ap=stage[:],
                            idxs_ap=idx_sb[:, col:col + HB // 16],
                            num_idxs=HB,
                            num_idxs_reg=HB,
                            elem_size=D,
                            queue_num=2 * h + qn % 2,
                            single_packet=False,
                        )
                        if e == 0:
                            nc.vector.tensor_copy(hv, g2v)
                        else:
                            nc.vector.tensor_add(hv, hv, g2v)
                        col += HB // 16
                        qn += 1
                    nc.sync.dma_start(
                        inb[:][h * HB:(h + 1) * HB, :]
                        .rearrange("(t p) d -> p t d", p=128), hv)
                    nc.gpsimd.collective_compute(
                        "AllGather",
                        _mb.AluOpType.bypass,
                        replica_groups=[list(range(N_CORES))],
                        ins=[inb[:][h * HB:(h + 1) * HB, :].opt()],
                        outs=[agh[h].opt()],
                    )

            # ---- transpose: outputs slice -> T (partition-major) ---------
            # T row for column l = (l%128)*98 + l//128; all 98 transposed
            # chunks accumulate in one SBUF tile, written with a single
            # 128x50KB-contiguous DMA on the scalar HWDGE ring.
            with tc.tile_pool(name="psum", bufs=4, space="PSUM") as pspool:
                for c in range(TCOLS // 128):
                    ps = pspool.tile([128, 128], f32)
                    nc.tensor.transpose(ps[:], ok_sb[:, c * 128:(c + 1) * 128],
                                        ident_sb[:])
                    nc.vector.tensor_copy(ok_sb[:, c * 128:(c + 1) * 128],
                                          ps[:])
                nc.scalar.dma_start(
                    tdram[:].rearrange("(p c) d -> p c d", p=128),
                    ok_sb[:].rearrange("p (c d) -> p c d", d=D))


            # ---- Phase B: sample gathers + dot products ------------------
            # samples sorted by (b-half, b, s); per half: 2 quarter calls
            vals_sb = vpool.tile([128, NPB // 128], f32)
            PQ = NPB // 4                 # 3200 per quarter-call
            with (
                tc.tile_pool(name="gb", bufs=2) as gbpool,
                tc.tile_pool(name="ib", bufs=2) as ibpool,
            ):
                gcol = col
                icol = col + 4 * (PQ // 16)
                for u in range(4):        # quarter u; half = u // 2
                    gt2 = gbpool.tile([128, (PQ // 128) * D], f32)
                    it2 = ibpool.tile([128, (PQ // 128) * D], f32)
                    nc.gpsimd.dma_gather(
                        out_ap=gt2[:].rearrange("p (c d) -> p c d", d=D),
                        in_ap=tdram[:],
                        idxs_ap=idx_sb[:, gcol:gcol + PQ // 16],
                        num_idxs=PQ,
                        num_idxs_reg=PQ,
                        elem_size=D,
                        queue_num=u % 2,
                        single_packet=False,
                    )
                    nc.gpsimd.dma_gather(
                        out_ap=it2[:].rearrange("p (c d) -> p c d", d=D),
                        in_ap=agh[u // 2][:],
                        idxs_ap=idx_sb[:, icol:icol + PQ // 16],
                        num_idxs=PQ,
                        num_idxs_reg=PQ,
                        elem_size=D,
                        queue_num=2 + u % 2,
                        single_packet=False,
                    )
                    nc.vector.tensor_mul(gt2[:], gt2[:], it2[:])
                    nc.vector.reduce_sum(
                        vals_sb[:, u * (PQ // 128):(u + 1) * (PQ // 128)],
                        gt2[:].rearrange("p (c d) -> p c d", d=D),
                        axis=_mb.AxisListType.X)
                    gcol += PQ // 16
                    icol += PQ // 16

            nc.sync.dma_start(vals_d[:], vals_sb[:])

    nc.compile()
    return nc


def _get_nc():
    global _nc_cache
    if _nc_cache is None:
        _nc_cache = _build_nc()
    return _nc_cache


def _wrap16(flat):
    """[n] int array (n % 16 == 0) -> [128, n//16] int16 laid out as the
    dma_gather ucode reads it: idx j at (partition j%16, col j//16),
    replicated across the eight 16-partition groups."""
    m = np.asarray(flat, dtype=np.int16).reshape(-1, 16).T  # [16, n//16]
    return np.tile(m, (8, 1))


def _prepare_core(k, doc_ids, context_ids, sample_ids):
    """Host-side index prep for core k. Returns (idx_all, bbp, ssp, valid)."""
    bsl = slice(k * BL, (k + 1) * BL)
    doc = np.asarray(doc_ids[bsl], dtype=np.int64)          # [BL]
    ctx = np.asarray(context_ids[bsl], dtype=np.int64)      # [BL, CTX]
    HBH = BL // 2

    segs = []
    g2_segs = []
    for h in range(2):
        hsl = slice(h * HBH, (h + 1) * HBH)
        doc_h = doc[hsl]
        ctx_h = ctx[hsl]
        stage_pos = np.empty((HBH, 9), dtype=np.int64)
        srow = 0
        doc_w = doc_h // WIN
        for w in range(NWIN):
            sel = np.nonzero(doc_w == w)[0]
            uniq, inv = np.unique(doc_h[sel] - w * WIN, return_inverse=True)
            n = len(uniq)
            if n > NP_DOC:
                raise ValueError(f"core {k}: doc window {h}/{w} overflow ({n})")
            lst = np.zeros(NP_DOC, dtype=np.int64)
            lst[:n] = uniq
            segs.append(_wrap16(lst))
            j = inv
            stage_pos[sel, 0] = (srow + (j % 128) * (NP_DOC // 128)
                                 + j // 128)
            srow += NP_DOC
        ctx_w = ctx_h // WIN
        CQ = NP_CTX // 2
        for w in range(NWIN):
            bb_, cc_ = np.nonzero(ctx_w == w)
            uniq, inv = np.unique(ctx_h[bb_, cc_] - w * WIN,
                                  return_inverse=True)
            n = len(uniq)
            if n > NP_CTX:
                raise ValueError(f"core {k}: ctx window {h}/{w} overflow ({n})")
            lst = np.zeros(NP_CTX, dtype=np.int64)
            lst[:n] = uniq
            for q in range(2):
                segs.append(_wrap16(lst[q * CQ:(q + 1) * CQ]))
            j = inv
            q_, jq = j // CQ, j % CQ
            stage_pos[bb_, cc_ + 1] = (srow + q_ * CQ
                                       + (jq % 128) * (CQ // 128) + jq // 128)
            srow += NP_CTX
        # g2: one call per entry e, 1024 slots in batch order
        for e in range(9):
            segs.append(_wrap16(stage_pos[:, e]))
    # phase B: samples sorted by (b-half, b, s); each half padded to NPB/2
    smp = np.asarray(sample_ids, dtype=np.int64)            # [B, S]
    bb, ss = np.nonzero(smp // RV == k)
    half = (bb % BL) // HBH
    order = np.argsort(half, kind="stable")
    bb, ss, half = bb[order], ss[order], half[order]
    NH = NPB // 2
    bbp = np.zeros(NPB, dtype=np.int64)
    ssp = np.zeros(NPB, dtype=np.int64)
    valid = np.zeros(NPB, dtype=bool)
    gi = np.zeros(NPB, dtype=np.int64)
    ii = np.zeros(NPB, dtype=np.int64)
    for h in range(2):
        sel = half == h
        n_h = int(sel.sum())
        if n_h > NH:
            raise ValueError(f"core {k}: sample half {h} overflow ({n_h})")
        sl = slice(h * NH, h * NH + n_h)
        bbp[sl], ssp[sl], valid[sl] = bb[sel], ss[sel], True
        lcol = smp[bb[sel], ss[sel]] - k * RV
        gi[sl] = (lcol % 128) * (TCOLS // 128) + lcol // 128
        ii[sl] = (bb[sel] // BL) * HBH + (bb[sel] % HBH)
    PQ = NPB // 4
    for u in range(4):
        segs.append(_wrap16(gi[u * PQ:(u + 1) * PQ]))
    for u in range(4):
        segs.append(_wrap16(ii[u * PQ:(u + 1) * PQ]))

    idx_all = np.concatenate(segs, axis=1)
    assert idx_all.shape == (128, IDX_COLS), idx_all.shape
    return idx_all, bbp, ssp, valid


def _run(doc_ids, context_ids, sample_ids, paragraph_matrix, word_matrix,
         outputs, trace=False):
    _install_ntff_hook()
    from concourse.bass_utils import run_bass_kernel_spmd

    nc = _get_nc()

    ptab = np.ascontiguousarray(np.asarray(paragraph_matrix, dtype=np.float32))
    wtab = np.ascontiguousarray(np.asarray(word_matrix, dtype=np.float32))
    outs = np.asarray(outputs, dtype=np.float32)
    ident = np.eye(128, dtype=np.float32)

    in_maps = []
    scatter = []
    for k in range(N_CORES):
        idx_all, bbp, ssp, valid = _prepare_core(k, doc_ids, context_ids,
                                                 sample_ids)
        oc = np.zeros((128, TCOLS), dtype=np.float32)
        oc[:, :RV] = outs[:, k * RV:(k + 1) * RV]
        in_maps.append({
            "idx": idx_all,
            "ptab": ptab,
            "wtab": wtab,
            "ocols": oc,
            "ident": ident,
        })
        scatter.append((bbp, ssp, valid))

    res = run_bass_kernel_spmd(nc, in_maps, core_ids=list(range(N_CORES)),
                               trace=trace)

    logits = np.zeros((B, S), dtype=np.float32)
    for k in range(N_CORES):
        bbp, ssp, valid = scatter[k]
        vals = res.results[k]["vals"]                       # [128, NPB//128]
        flat = vals.T.reshape(-1)                           # j = c*128 + p
        logits[bbp[valid], ssp[valid]] = flat[valid]
    return logits, res


def kernel(doc_ids, context_ids, sample_ids, paragraph_matrix, word_matrix,
           outputs):
    logits, _ = _run(doc_ids, context_ids, sample_ids, paragraph_matrix,
                     word_matrix, outputs, trace=False)
    return logits


def kernel_traced(doc_ids, context_ids, sample_ids, paragraph_matrix,
                  word_matrix, outputs):
    """Same as kernel() but captures an NTFF profile; returns
    (logits, exec_time_ns)."""
    logits, res = _run(doc_ids, context_ids, sample_ids, paragraph_matrix,
                       word_matrix, outputs, trace=True)
    return logits, res.exec_time_ns


# revision 29
# speedup vs baseline: 1.1017x; 1.1017x over previous
"""Distributed embedding-lookup kernel for 8 TRN2 NeuronCores (Bass/Tile).

Computes, for full inputs:
    word_sum = sum(word_matrix[context_ids], axis=1)        # [B, D]
    inputs   = paragraph_matrix[doc_ids] + word_sum         # [B, D]
    out_cols = outputs[:, sample_ids]                       # [D, B, S]
    logits   = einsum("bd,dbs->bs", inputs, out_cols)       # [B, S]

Strategy (SPMD, one NEFF on 8 cores; per-core variation lives in input data):
  Phase A (batch-sharded, 2048 rows/core): all 9 embedding-row fetches per
    batch element (1 doc + 8 ctx) are gathered via windowed dma_gather
    (int16 indices limited to 32767 -> 4 windows of 25000 rows per table),
    written compacted to a DRAM stage buffer, re-gathered in
    (entry-major, batch-minor) slot order (stage row ids < 32767), then
    reduced over the 9 entries with strided DVE adds -> inputs [2048, 128].
  AllGather inputs across cores -> [16384, 128] per core.
  Phase B (vocab-sharded: core k owns outputs[:, 12500k:12500(k+1)]):
    PE-transpose the slice to T [12544, 128] in DRAM; dma_gather T rows by
    local sample column and inputs rows by sample batch id; DVE mul +
    free-dim reduce gives one dot product per sample; host scatters values
    into the [16384, 6] output.
All index lists / stage positions / scatter maps are precomputed on host
(pure index arithmetic; all bulk data movement happens on device).
"""

import sys
import types

import numpy as np

# ---------------------------------------------------------------------------
# problem constants (hardcoded per contract)
B = 16384
D = 128
CTX = 8
S = 6
V = 100000
N_CORES = 8
BL = B // N_CORES              # 2048 batch rows per core
RV = V // N_CORES              # 12500 outputs columns per core
WIN = 25000                    # gather window (int16 indices must be <= 32767)
NWIN = V // WIN                # 4 windows per table
NP_DOC = 384                   # per-(half,window) doc list (avg 256, seed max 299)
NP_CTX = 2304                  # per-(half,window) ctx list (avg 2048, seed max 2081)
NSTAGE = NWIN * (NP_DOC + NP_CTX)   # 10752 stage rows per half
NPB = 12800                    # padded per-core samples (avg 12288; 6400/half, seed max 6261)
TCOLS = 12544                  # outputs cols padded to 98*128 for transpose
IDX_COLS = (2 * NWIN * (NP_DOC // 16) + 2 * NWIN * (NP_CTX // 16)
            + 18 * (BL // 2 // 16) + 8 * (NPB // 4 // 16))  # 4096

_nc_cache = None


def _install_ntff_hook():
    """antenv.axon_hooks is absent from this image; inject it so
    run_bass_kernel_spmd(trace=True) can capture NTFF profiles."""
    if "antenv.axon_hooks" in sys.modules:
        return
    mod = types.ModuleType("antenv.axon_hooks")
    mod._hook = None
    mod.set_axon_ntff_profile_hook = lambda h: setattr(mod, "_hook", h)
    mod.get_axon_ntff_profile_hook = lambda: mod._hook
    sys.modules["antenv.axon_hooks"] = mod
    try:
        import antenv
        antenv.axon_hooks = mod
        from trn_agent_boot.trn_boot import _ntff_profile_via_ctypes
        mod.set_axon_ntff_profile_hook(
            _ntff_profile_via_ctypes("/opt/axon/libaxon_pjrt.so"))
    except Exception:
        pass


def _patch_swdge_lane_assignment():
    """Tile round-robins SWDGE DMA completion sems over all 8 DMASW lanes,
    but the runtime locks each sem lane to the first SWDGE queue that
    increments it — mixed-queue kernels then abort.  Pin queue-tagged SWDGE
    ops (dma_gather et al.) to lane == queue_num, and round-robin untagged
    SWDGE DMAs over lanes 4..7 so the two sets never share a lane."""
    import concourse.tile_sem_assignment as tsa
    import concourse.mybir as mybir
    from concourse import bass_isa

    if getattr(tsa.TileClockTick, "_lane_patch", False):
        return
    orig = tsa.TileClockTick._assign_tick

    def _assign_tick(self, inst):
        if (
            isinstance(inst, tsa.DMAInst)
            and not isinstance(inst, bass_isa.UserSyncedRemoteDMADescs)
            and inst.engine == mybir.EngineType.Pool
        ):
            qn = getattr(inst, "queue_num", None)
            if isinstance(qn, int) and 0 <= qn <= 3:
                lane = qn
            else:
                lane = 4 + self.next_sw_dma_idx % 4
                self.next_sw_dma_idx += 1
            proc = tsa.PROC_NAME_TO_IDX[f"DMASW{lane}"]
            inst.bass_scheduled_tick = self.global_clock.advance(proc)
            inst.bass_scheduled_proc = proc
            inst.bass_scheduled_scope = self.scope_name
            self._proc_insts[self.root_scope_name][proc].append(inst)
            eng_proc = tsa.ENGINE_TO_IDX[inst.engine]
            if getattr(inst, "gen_mode", 0) == 1 and proc != eng_proc:
                eng_tick = self.global_clock.advance(eng_proc)
                self.tc.prep_eng_ticks[inst.name] = (eng_proc, eng_tick)
                self._prep_eng_names[self.root_scope_name].append(inst.name)
            return
        return orig(self, inst)

    tsa.TileClockTick._assign_tick = _assign_tick
    tsa.TileClockTick._lane_patch = True


def _build_nc():
    import concourse.bacc as bacc
    import concourse.mybir as mybir
    import concourse.tile as tile

    _patch_swdge_lane_assignment()

    f32 = mybir.dt.float32
    i16 = mybir.dt.int16

    nc = bacc.Bacc("TRN2", target_bir_lowering=False, debug=False,
                   num_devices=N_CORES, num_swdge_queues=4)

    idx_d = nc.dram_tensor("idx", [128, IDX_COLS], i16, kind="ExternalInput")
    ptab = nc.dram_tensor("ptab", [V, D], f32, kind="ExternalInput")
    wtab = nc.dram_tensor("wtab", [V, D], f32, kind="ExternalInput")
    ocols = nc.dram_tensor("ocols", [128, TCOLS], f32, kind="ExternalInput")
    ident = nc.dram_tensor("ident", [128, 128], f32, kind="ExternalInput")
    vals_d = nc.dram_tensor("vals", [128, NPB // 128], f32,
                            kind="ExternalOutput")

    with tile.TileContext(nc) as tc:
        with (
            tc.tile_pool(name="dram", bufs=1, space="DRAM") as dpool,
            tc.tile_pool(name="const", bufs=1) as cpool,
            tc.tile_pool(name="acc", bufs=1) as apool,
            tc.tile_pool(name="vals", bufs=1) as vpool,
        ):
            stage0 = dpool.tile([NSTAGE, D], f32)
            stage1 = dpool.tile([NSTAGE, D], f32)
            stages = [stage0, stage1]
            tdram = dpool.tile([TCOLS, D], f32)
            inb = dpool.tile([BL, D], f32)
            agh0 = dpool.tile([B // 2, D], f32)
            agh1 = dpool.tile([B // 2, D], f32)
            agh = [agh0, agh1]

            import concourse.mybir as _mb

            idx_sb = cpool.tile([128, IDX_COLS], i16)
            nc.sync.dma_start(idx_sb[:], idx_d[:])
            ident_sb = cpool.tile([128, 128], f32)
            nc.sync.dma_start(ident_sb[:], ident[:])
            ok_sb = cpool.tile([128, TCOLS], f32)
            nc.scalar.dma_start(ok_sb[:], ocols[:])

            # ---- Phase A, by batch-half: windowed gathers -> stage_h,
            # slot-order regather + entry reduction; each half's AllGather
            # fires while the other half is still gathering.
            HB = BL // 2                  # 1024 rows per half
            col = 0
            qn = 0
            acc = apool.tile([128, (BL // 128) * D], f32)
            acc3 = acc[:].rearrange("p (t d) -> p t d", d=D)
            with (
                tc.tile_pool(name="g1doc", bufs=4) as gdoc,
                tc.tile_pool(name="g1ctx", bufs=6) as gctx,
                tc.tile_pool(name="g2", bufs=6) as g2pool,
            ):
                for h in range(2):
                    stage = stages[h]
                    srow = 0
                    for w in range(NWIN):
                        gt = gdoc.tile([128, NP_DOC // 128 * D], f32)
                        gt3 = gt[:].rearrange("p (c d) -> p c d", d=D)
                        nc.gpsimd.dma_gather(
                            out_ap=gt3,
                            in_ap=ptab[w * WIN:(w + 1) * WIN, :],
                            idxs_ap=idx_sb[:, col:col + NP_DOC // 16],
                            num_idxs=NP_DOC,
                            num_idxs_reg=NP_DOC,
                            elem_size=D,
                            queue_num=qn % 4,
                            single_packet=False,
                        )
                        nc.sync.dma_start(
                            stage[:][srow:srow + NP_DOC, :]
                            .rearrange("(p c) d -> p c d", p=128),
                            gt3)
                        col += NP_DOC // 16
                        srow += NP_DOC
                        qn += 1
                    CQ = NP_CTX // 2      # 1152 per sub-call
                    for w in range(NWIN):
                        for q in range(2):
                            gt = gctx.tile([128, CQ // 128 * D], f32)
                            gt3 = gt[:].rearrange("p (c d) -> p c d", d=D)
                            nc.gpsimd.dma_gather(
                                out_ap=gt3,
                                in_ap=wtab[w * WIN:(w + 1) * WIN, :],
                                idxs_ap=idx_sb[:, col:col + CQ // 16],
                                num_idxs=CQ,
                                num_idxs_reg=CQ,
                                elem_size=D,
                                queue_num=qn % 4,
                                single_packet=False,
                            )
                            nc.sync.dma_start(
                                stage[:][srow:srow + CQ, :]
                                .rearrange("(p c) d -> p c d", p=128),
                                gt3)
                            col += CQ // 16
                            srow += CQ
                            qn += 1
                    hv = acc3[:, h * (HB // 128):(h + 1) * (HB // 128)]
                    for e in range(9):
                        g2t = g2pool.tile([128, (HB // 128) * D], f32)
                        g2v = g2t[:].rearrange("p (t d) -> p t d", d=D)
                        nc.gpsimd.dma_gather(
                            out_ap=g2v,
                            in_ap=stage[:],
                            idxs_ap=idx_sb[:, col:col + HB // 16],
                            num_idxs=HB,
                            num_idxs_reg=HB,
                            elem_size=D,
                            queue_num=qn % 4,
                            single_packet=False,
                        )
                        if e == 0:
                            nc.vector.tensor_copy(hv, g2v)
                        else:
                            nc.vector.tensor_add(hv, hv, g2v)
                        col += HB // 16
                        qn += 1
                    nc.sync.dma_start(
                        inb[:][h * HB:(h + 1) * HB, :]
                        .rearrange("(t p) d -> p t d", p=128), hv)
                    nc.gpsimd.collective_compute(
                        "AllGather",
                        _mb.AluOpType.bypass,
                        replica_groups=[list(range(N_CORES))],
                        ins=[inb[:][h * HB:(h + 1) * HB, :].opt()],
                        outs=[agh[h].opt()],
                    )

            # ---- transpose: outputs slice -> T (partition-major) ---------
            # T row for column l = (l%128)*98 + l//128; all 98 transposed
            # chunks accumulate in one SBUF tile, written with a single
            # 128x50KB-contiguous DMA on the scalar HWDGE ring.
            with tc.tile_pool(name="psum", bufs=4, space="PSUM") as pspool:
                for c in range(TCOLS // 128):
                    ps = pspool.tile([128, 128], f32)
                    nc.tensor.transpose(ps[:], ok_sb[:, c * 128:(c + 1) * 128],
                                        ident_sb[:])
                    nc.vector.tensor_copy(ok_sb[:, c * 128:(c + 1) * 128],
                                          ps[:])
                nc.scalar.dma_start(
                    tdram[:].rearrange("(p c) d -> p c d", p=128),
                    ok_sb[:].rearrange("p (c d) -> p c d", d=D))


            # ---- Phase B: sample gathers + dot products ------------------
            # samples sorted by (b-half, b, s); per half: 2 quarter calls
            vals_sb = vpool.tile([128, NPB // 128], f32)
            PQ = NPB // 4                 # 3200 per quarter-call
            with (
                tc.tile_pool(name="gb", bufs=2) as gbpool,
                tc.tile_pool(name="ib", bufs=2) as ibpool,
            ):
                gcol = col
                icol = col + 4 * (PQ // 16)
                for u in range(4):        # quarter u; half = u // 2
                    gt2 = gbpool.tile([128, (PQ // 128) * D], f32)
                    it2 = ibpool.tile([128, (PQ // 128) * D], f32)
                    nc.gpsimd.dma_gather(
                        out_ap=gt2[:].rearrange("p (c d) -> p c d", d=D),
                        in_ap=tdram[:],
                        idxs_ap=idx_sb[:, gcol:gcol + PQ // 16],
                        num_idxs=PQ,
                        num_idxs_reg=PQ,
                        elem_size=D,
                        queue_num=u,
                        single_packet=False,
                    )
                    nc.gpsimd.dma_gather(
                        out_ap=it2[:].rearrange("p (c d) -> p c d", d=D),
                        in_ap=agh[u // 2][:],
                        idxs_ap=idx_sb[:, icol:icol + PQ // 16],
                        num_idxs=PQ,
                        num_idxs_reg=PQ,
                        elem_size=D,
                        queue_num=(u + 2) % 4,
                        single_packet=False,
                    )
                    nc.vector.tensor_mul(gt2[:], gt2[:], it2[:])
                    nc.vector.reduce_sum(
                        vals_sb[:, u * (PQ // 128):(u + 1) * (PQ // 128)],
                        gt2[:].rearrange("p (c d) -> p c d", d=D),
                        axis=_mb.AxisListType.X)
                    gcol += PQ // 16
                    icol += PQ // 16

            nc.sync.dma_start(vals_d[:], vals_sb[:])

    nc.compile()
    return nc


def _get_nc():
    global _nc_cache
    if _nc_cache is None:
        _nc_cache = _build_nc()
    return _nc_cache


def _wrap16(flat):
    """[n] int array (n % 16 == 0) -> [128, n//16] int16 laid out as the
    dma_gather ucode reads it: idx j at (partition j%16, col j//16),
    replicated across the eight 16-partition groups."""
    m = np.asarray(flat, dtype=np.int16).reshape(-1, 16).T  # [16, n//16]
    return np.tile(m, (8, 1))


def _prepare_core(k, doc_ids, context_ids, sample_ids):
    """Host-side index prep for core k. Returns (idx_all, bbp, ssp, valid)."""
    bsl = slice(k * BL, (k + 1) * BL)
    doc = np.asarray(doc_ids[bsl], dtype=np.int64)          # [BL]
    ctx = np.asarray(context_ids[bsl], dtype=np.int64)      # [BL, CTX]
    HBH = BL // 2

    segs = []
    g2_segs = []
    for h in range(2):
        hsl = slice(h * HBH, (h + 1) * HBH)
        doc_h = doc[hsl]
        ctx_h = ctx[hsl]
        stage_pos = np.empty((HBH, 9), dtype=np.int64)
        srow = 0
        doc_w = doc_h // WIN
        for w in range(NWIN):
            sel = np.nonzero(doc_w == w)[0]
            uniq, inv = np.unique(doc_h[sel] - w * WIN, return_inverse=True)
            n = len(uniq)
            if n > NP_DOC:
                raise ValueError(f"core {k}: doc window {h}/{w} overflow ({n})")
            lst = np.zeros(NP_DOC, dtype=np.int64)
            lst[:n] = uniq
            segs.append(_wrap16(lst))
            j = inv
            stage_pos[sel, 0] = (srow + (j % 128) * (NP_DOC // 128)
                                 + j // 128)
            srow += NP_DOC
        ctx_w = ctx_h // WIN
        CQ = NP_CTX // 2
        for w in range(NWIN):
            bb_, cc_ = np.nonzero(ctx_w == w)
            uniq, inv = np.unique(ctx_h[bb_, cc_] - w * WIN,
                                  return_inverse=True)
            n = len(uniq)
            if n > NP_CTX:
                raise ValueError(f"core {k}: ctx window {h}/{w} overflow ({n})")
            lst = np.zeros(NP_CTX, dtype=np.int64)
            lst[:n] = uniq
            for q in range(2):
                segs.append(_wrap16(lst[q * CQ:(q + 1) * CQ]))
            j = inv
            q_, jq = j // CQ, j % CQ
            stage_pos[bb_, cc_ + 1] = (srow + q_ * CQ
                                       + (jq % 128) * (CQ // 128) + jq // 128)
            srow += NP_CTX
        # g2: one call per entry e, 1024 slots in batch order
        for e in range(9):
            segs.append(_wrap16(stage_pos[:, e]))
    # phase B: samples sorted by (b-half, b, s); each half padded to NPB/2
    smp = np.asarray(sample_ids, dtype=np.int64)            # [B, S]
    bb, ss = np.nonzero(smp // RV == k)
    half = (bb % BL) // HBH
    order = np.argsort(half, kind="stable")
    bb, ss, half = bb[order], ss[order], half[order]
    NH = NPB // 2
    bbp = np.zeros(NPB, dtype=np.int64)
    ssp = np.zeros(NPB, dtype=np.int64)
    valid = np.zeros(NPB, dtype=bool)
    gi = np.zeros(NPB, dtype=np.int64)
    ii = np.zeros(NPB, dtype=np.int64)
    for h in range(2):
        sel = half == h
        n_h = int(sel.sum())
        if n_h > NH:
            raise ValueError(f"core {k}: sample half {h} overflow ({n_h})")
        sl = slice(h * NH, h * NH + n_h)
        bbp[sl], ssp[sl], valid[sl] = bb[sel], ss[sel], True
        lcol = smp[bb[sel], ss[sel]] - k * RV
        gi[sl] = (lcol % 128) * (TCOLS // 128) + lcol // 128
        ii[sl] = (bb[sel] // BL) * HBH + (bb[sel] % HBH)
    PQ = NPB // 4
    for u in range(4):
        segs.append(_wrap16(gi[u * PQ:(u + 1) * PQ]))
    for u in range(4):
        segs.append(_wrap16(ii[u * PQ:(u + 1) * PQ]))

    idx_all = np.concatenate(segs, axis=1)
    assert idx_all.shape == (128, IDX_COLS), idx_all.shape
    return idx_all, bbp, ssp, valid


def _run(doc_ids, context_ids, sample_ids, paragraph_matrix, word_matrix,
         outputs, trace=False):
    _install_ntff_hook()
    from concourse.bass_utils import run_bass_kernel_spmd

    nc = _get_nc()

    ptab = np.ascontiguousarray(np.asarray(paragraph_matrix, dtype=np.float32))
    wtab = np.ascontiguousarray(np.asarray(word_matrix, dtype=np.float32))
    outs = np.asarray(outputs, dtype=np.float32)
    ident = np.eye(128, dtype=np.float32)

    in_maps = []
    scatter = []
    for k in range(N_CORES):
        idx_all, bbp, ssp, valid = _prepare_core(k, doc_ids, context_ids,
                                                 sample_ids)
        oc = np.zeros((128, TCOLS), dtype=np.float32)
        oc[:, :RV] = outs[:, k * RV:(k + 1) * RV]
        in_maps.append({
            "idx": idx_all,
            "ptab": ptab,
            "wtab": wtab,
            "ocols": oc,
            "ident": ident,
        })
        scatter.append((bbp, ssp, valid))

    res = run_bass_kernel_spmd(nc, in_maps, core_ids=list(range(N_CORES)),
                               trace=trace)

    logits = np.zeros((B, S), dtype=np.float32)
    for k in range(N_CORES):
        bbp, ssp, valid = scatter[k]
        vals = res.results[k]["vals"]                       # [128, NPB//128]
        flat = vals.T.reshape(-1)                           # j = c*128 + p
        logits[bbp[valid], ssp[valid]] = flat[valid]
    return logits, res


def kernel(doc_ids, context_ids, sample_ids, paragraph_matrix, word_matrix,
           outputs):
    logits, _ = _run(doc_ids, context_ids, sample_ids, paragraph_matrix,
                     word_matrix, outputs, trace=False)
    return logits


def kernel_traced(doc_ids, context_ids, sample_ids, paragraph_matrix,
                  word_matrix, outputs):
    """Same as kernel() but captures an NTFF profile; returns
    (logits, exec_time_ns)."""
    logits, res = _run(doc_ids, context_ids, sample_ids, paragraph_matrix,
                       word_matrix, outputs, trace=True)
    return logits, res.exec_time_ns
